# revision 4
# baseline (speedup 1.0000x reference)
"""Block-diagonal masked dense + BatchNorm(train) + ReLU on 8 TRN2 NeuronCores.

Math: out = x @ (W * blockdiag_mask) + bias; BN over batch; relu.
The mask keeps 64 diagonal blocks of shape [64 in, 64 out]. Group g only
couples x[:, 64g:64g+64] to out[:, 64g:64g+64].

Sharding: groups are split across cores (8 groups per core). Each core owns a
disjoint 512-column slice of both input and output features, so the matmul and
the per-feature batch statistics are fully core-local (no collectives).

Per-core device program (all shapes hardcoded, fp16 I/O — the 2e-2 harness
gate leaves ~40x margin over fp16's ~5e-4 rounding error, and halving the
bytes halves the HBM-bound runtime):
  inputs:  xT [128, 4, 4096] fp16 (x slice transposed + chunk-permuted on
           host so every DMA is 128 contiguous rows), wd [128, 4, 128] fp16
           (per 128-row chunk a 2x2 block-diagonal of two 64x64 group
           blocks, host-permuted), gb [128, 8] f32 (gamma | beta chunks)
  output:  yT [128, 4, 4096] fp16 (host transposes back and upcasts)
  phase 1: per chunk c and batch tile t: one K=128 fp16 matmul
           (block-diagonal zeros kill cross-group terms); DVE
           bn_stats/bn_aggr give mean/var per output feature.
  coefs:   A = gamma * rsqrt(var + eps); B = beta - mean * A.
           (bias cancels in BN: out and mean(out) shift equally, and
           variance is bias-invariant, so bias never reaches the device.)
  phase 2: recompute the matmul (x stays SBUF-resident; the only engines
           that can read PSUM are DVE and ACT, both already saturated, so
           a spill pass would cost more than PE recompute) and apply
           relu(psum * A + B) in one ScalarE pass, PSUM -> SBUF(fp16).

Scheduling (the critical path is DVE's ~22us of serialized bn_stats — every
downstream stage of the last chunk appends to it):
  - PE emits ALL phase-1 matmuls with priority; phase-2 recompute megas are
    interleaved only where they fill PE gaps. This keeps stats(c+1) from
    waiting on coefs(c) (the round-2 mistake, worth ~6us of DVE idle).
  - The last chunk's phase 2 is fine-grained: per-mega stores, relus split
    between ACT (PSUM relu) and DVE (tensor_scalar mult/add + max pair) so
    the post-stats tail is ~2 megas long instead of 4.
  - Input DMAs split across both HWDGE queues (Sync: wd + chunks 0-1,
    Scalar: gb + chunks 2-3) so descriptor generation parallelizes; output
    stores ride Sync behind the inputs.
  - Tiny per-chunk coefficient muls run on Pool; sqrt(c) is emitted right
    before the relus that consume it, after aggr(c) is long done.

Accuracy: ~5e-4 rel L2 vs the fp32 reference (fp16 rounding of x, W and y;
BN statistics run in fp32 from the f32 PSUM accumulators).
"""

import numpy as np

import concourse.bass as bass
import concourse.tile as tile
from concourse import mybir
from concourse.bass_utils import run_bass_kernel_spmd

F32 = mybir.dt.float32
F16 = mybir.dt.float16

NCORES = 8
BATCH = 4096
DIM = 4096
DCORE = DIM // NCORES          # 512 features per core
CHUNKS = DCORE // 128          # 4 partition chunks (2 groups each)
BTILE = 512                    # matmul moving tile (one PSUM bank, fp32)
BTILES = BATCH // BTILE        # 8
MEGA = 1024                    # PSUM mega-tile free dim (2 banks, 2 matmuls)
MEGAS = BATCH // MEGA          # 4 per chunk
EPS = 1e-3

_MAX_WAITS = 1


def _split_multi_waits(nc: bass.Bass, max_waits: int = _MAX_WAITS) -> None:
    # The walrus build in this container rejects instructions carrying more
    # than one sync-wait command (any engine, any opcode). Hoist extra waits
    # onto same-engine NOPs inserted immediately before the instruction —
    # identical semantics, since the engine blocks on each wait in order.
    # Snapshot every block BEFORE creating any nop: the engine builders append
    # new instructions to the current (last) block as a side effect, and the
    # final wholesale reassignment below discards those spurious appends.
    snapshots = [
        (bb, list(bb.instructions)) for f in nc.m.functions for bb in f.blocks
    ]
    rebuilt = []
    for bb, insts in snapshots:
        new = []
        for ins in insts:
            si = getattr(ins, "sync_info", None)
            waits = list(si.on_wait) if si is not None and si.on_wait else []
            if len(waits) > max_waits:
                head = waits[:-max_waits]
                for i in range(0, len(head), max_waits):
                    nop = nc.engines[ins.engine].nop().ins
                    nop.sync_info = mybir.SyncInfo(
                        on_wait=head[i : i + max_waits], on_update=[]
                    )
                    new.append(nop)
                ins.sync_info = mybir.SyncInfo(
                    on_wait=waits[-max_waits:],
                    on_update=list(si.on_update or []),
                )
            new.append(ins)
        rebuilt.append((bb, new))
    for bb, new in rebuilt:
        bb.instructions = new


def _build_nc() -> bass.Bass:
    nc = bass.Bass()
    xT = nc.dram_tensor("xT", [128, CHUNKS, BATCH], F16, kind="ExternalInput")
    wd = nc.dram_tensor("wd", [128, CHUNKS, 128], F16, kind="ExternalInput")
    gb = nc.dram_tensor("gb", [128, 2 * CHUNKS], F32, kind="ExternalInput")
    yT = nc.dram_tensor("yT", [128, CHUNKS, BATCH], F16, kind="ExternalOutput")

    with tile.TileContext(nc) as tc:
        with (
            tc.tile_pool(name="singles", bufs=1) as singles,
            tc.tile_pool(name="stats", bufs=1) as statp,
            tc.tile_pool(name="psum1", bufs=4, space="PSUM") as psum1,
            tc.tile_pool(name="psum2", bufs=2, space="PSUM") as psum2,
            tc.tile_pool(name="y", bufs=3) as ypool,
        ):
            # Input issue split across both HWDGE queues so descriptor
            # generation (~0.7us per DMA) parallelizes. wd first on Sync:
            # it is tiny and gates the first matmul.
            xsb = singles.tile([128, CHUNKS, BATCH], F16)
            wsb = singles.tile([128, CHUNKS, 128], F16)
            gbsb = singles.tile([128, 2 * CHUNKS], F32)
            nc.sync.dma_start(wsb[:], wd[:, :, :])
            for h in range(2):
                sl = bass.ds(h * (BATCH // 2), BATCH // 2)
                nc.sync.dma_start(xsb[:, 0, sl], xT[:, 0, sl])
            nc.scalar.dma_start(gbsb[:], gb[:, :])
            nc.scalar.dma_start(xsb[:, 2, :], xT[:, 2, :])
            nc.sync.dma_start(xsb[:, 1, :], xT[:, 1, :])
            nc.scalar.dma_start(xsb[:, 3, :], xT[:, 3, :])
            epsb = singles.tile([128, 1], F32)
            nc.vector.memset(epsb[:], EPS)

            stats = statp.tile([128, CHUNKS, BTILES, 6], F32)
            mv = statp.tile([128, CHUNKS, 2], F32)
            coefA = statp.tile([128, CHUNKS], F32)
            coefB = statp.tile([128, CHUNKS], F32)
            tmp = statp.tile([128, CHUNKS], F32)
            tmp2 = statp.tile([128, CHUNKS], F32)

            def one_matmul(ps, os, c: int, t: int):
                nc.tensor.matmul(
                    ps[:, os],
                    lhsT=wsb[:, c, :],
                    rhs=xsb[:, c, bass.ds(t * BTILE, BTILE)],
                    start=True, stop=True,
                )

            def phase1_mm(c: int):
                for t in range(BTILES):
                    ps = psum1.tile([128, BTILE], F32, tag="ps1")
                    one_matmul(ps, slice(None), c, t)
                    nc.vector.bn_stats(stats[:, c, t, :], ps[:, :])
                nc.vector.bn_aggr(mv[:, c, :], stats[:, c, :, :])

            def coefs(c: int):
                nc.scalar.activation(
                    tmp[:, c : c + 1], mv[:, c, 1:2],
                    mybir.ActivationFunctionType.Sqrt,
                    bias=epsb[:], scale=1.0,
                )
                nc.vector.reciprocal(tmp[:, c : c + 1], tmp[:, c : c + 1])
                nc.gpsimd.tensor_mul(
                    coefA[:, c : c + 1], tmp[:, c : c + 1], gbsb[:, c : c + 1]
                )
                nc.gpsimd.tensor_mul(
                    tmp2[:, c : c + 1], mv[:, c, 0:1], coefA[:, c : c + 1]
                )
                nc.gpsimd.tensor_sub(
                    coefB[:, c : c + 1],
                    gbsb[:, CHUNKS + c : CHUNKS + c + 1],
                    tmp2[:, c : c + 1],
                )

            def phase2_chunk(c: int):
                # Two megas share one [128, 2048] staging tile => half as
                # many (bigger) output DMAs, issued on Sync.
                for half in range(2):
                    yt = ypool.tile([128, 2 * MEGA], F16, tag="yt")
                    for s in range(2):
                        m = half * 2 + s
                        ps = psum2.tile([128, MEGA], F32, tag="ps2")
                        for q in range(MEGA // BTILE):
                            one_matmul(
                                ps, bass.ds(q * BTILE, BTILE), c,
                                m * (MEGA // BTILE) + q,
                            )
                        nc.scalar.activation(
                            yt[:, bass.ds(s * MEGA, MEGA)], ps[:],
                            mybir.ActivationFunctionType.Relu,
                            bias=coefB[:, c : c + 1], scale=coefA[:, c : c + 1],
                        )
                    nc.sync.dma_start(
                        yT[:, c, bass.ds(half * 2 * MEGA, 2 * MEGA)], yt[:]
                    )

            def phase2_fine(c: int):
                # Last chunk: everything here is serially after the final
                # bn_stats, so shorten it. Per-mega stores; relus alternate
                # ACT (fused PSUM relu) and DVE (tensor_scalar pair), so the
                # two engines drain the four megas in parallel.
                for half in range(2):
                    yt = ypool.tile([128, 2 * MEGA], F16, tag="yt")
                    for s in range(2):
                        m = half * 2 + s
                        ys = yt[:, bass.ds(s * MEGA, MEGA)]
                        ps = psum2.tile([128, MEGA], F32, tag="ps2")
                        for q in range(MEGA // BTILE):
                            one_matmul(
                                ps, bass.ds(q * BTILE, BTILE), c,
                                m * (MEGA // BTILE) + q,
                            )
                        if s == 0:
                            nc.scalar.activation(
                                ys, ps[:],
                                mybir.ActivationFunctionType.Relu,
                                bias=coefB[:, c : c + 1],
                                scale=coefA[:, c : c + 1],
                            )
                        else:
                            nc.vector.tensor_scalar(
                                ys, ps[:],
                                coefA[:, c : c + 1], coefB[:, c : c + 1],
                                mybir.AluOpType.mult, mybir.AluOpType.add,
                            )
                            nc.vector.tensor_scalar_max(ys, ys, 0.0)
                        nc.sync.dma_start(
                            yT[:, c, bass.ds(m * MEGA, MEGA)], ys
                        )

            # Phase-1 matmuls (and with them the serialized DVE stats chain)
            # get strict priority on PE; phase-2 recompute fills the gaps.
            phase1_mm(0)
            phase1_mm(1)
            phase1_mm(2)
            coefs(0)
            phase2_chunk(0)
            phase1_mm(3)
            coefs(1)
            phase2_chunk(1)
            coefs(2)
            phase2_chunk(2)
            coefs(3)
            phase2_fine(3)
    _split_multi_waits(nc)
    return nc


_NC_CACHE: bass.Bass | None = None


def _get_nc() -> bass.Bass:
    global _NC_CACHE
    if _NC_CACHE is None:
        _NC_CACHE = _build_nc()
    return _NC_CACHE


def _make_in_maps(x, weight, gamma, beta):
    x16 = x.astype(np.float16)
    in_maps = []
    for c in range(NCORES):
        sl = slice(c * DCORE, (c + 1) * DCORE)
        # [p, ch, b] layout: partition p of chunk ch holds feature ch*128+p.
        xdev = np.ascontiguousarray(
            x16[:, sl].T.reshape(CHUNKS, 128, BATCH).transpose(1, 0, 2)
        )
        # Per 128-row chunk: [[w_{2g}, 0], [0, w_{2g+1}]] block-diagonal.
        wdc = np.zeros((DCORE, 128), np.float32)
        for g in range(DCORE // 64):
            r = slice(c * DCORE + g * 64, c * DCORE + (g + 1) * 64)
            col = (g % 2) * 64
            wdc[g * 64 : (g + 1) * 64, col : col + 64] = weight[r, r]
        wdev = np.ascontiguousarray(
            wdc.reshape(CHUNKS, 128, 128).transpose(1, 0, 2)
        ).astype(np.float16)
        gbdev = np.empty((128, 2 * CHUNKS), np.float32)
        gbdev[:, :CHUNKS] = gamma[sl].reshape(CHUNKS, 128).T
        gbdev[:, CHUNKS:] = beta[sl].reshape(CHUNKS, 128).T
        in_maps.append({"xT": xdev, "wd": wdev, "gb": gbdev})
    return in_maps


def kernel(x, weight, bias, gamma, beta, **_run_kwargs) -> np.ndarray:
    x = np.asarray(x, np.float32)
    weight = np.asarray(weight, np.float32)
    gamma = np.asarray(gamma, np.float32)
    beta = np.asarray(beta, np.float32)
    # bias is algebraically irrelevant: BN subtracts the batch mean, which
    # absorbs any constant per-feature shift, and variance is shift-invariant.

    nc = _get_nc()
    res = run_bass_kernel_spmd(
        nc, _make_in_maps(x, weight, gamma, beta),
        core_ids=list(range(NCORES)), **_run_kwargs,
    )
    out = np.empty((BATCH, DIM), np.float32)
    for c, r in enumerate(res.results):
        yc = r["yT"].transpose(1, 0, 2).reshape(DCORE, BATCH)
        out[:, c * DCORE : (c + 1) * DCORE] = yc.T.astype(np.float32)
    kernel.last_results = res
    return out


# revision 10
# speedup vs baseline: 1.1432x; 1.1432x over previous
"""Block-diagonal masked dense + BatchNorm(train) + ReLU on 8 TRN2 NeuronCores.

Math: out = x @ (W * blockdiag_mask) + bias; BN over batch; relu.
The mask keeps 64 diagonal blocks of shape [64 in, 64 out]. Group g only
couples x[:, 64g:64g+64] to out[:, 64g:64g+64].

Sharding: groups are split across cores (8 groups per core). Each core owns a
disjoint 512-column slice of both input and output features, so the matmul and
the per-feature batch statistics are fully core-local (no collectives).

Per-core device program (all shapes hardcoded, fp16 I/O — the 2e-2 harness
gate leaves ~40x margin over fp16's ~5e-4 rounding error, and halving the
bytes halves the HBM-bound runtime):
  inputs:  xT [128, 4, 4096] fp16 (x slice transposed + chunk-permuted on
           host so every DMA is 128 contiguous rows), wd [128, 4, 128] fp16
           (per 128-row chunk a 2x2 block-diagonal of two 64x64 group
           blocks, host-permuted), gb [128, 8] f32 (gamma | beta chunks)
  output:  yT [128, 4, 4096] fp16 (host transposes back and upcasts)
  phase 1: per chunk c and batch tile t: one K=128 fp16 matmul
           (block-diagonal zeros kill cross-group terms); DVE
           bn_stats/bn_aggr give mean/var per output feature.
  coefs:   A = gamma * rsqrt(var + eps); B = beta - mean * A.
           (bias cancels in BN: out and mean(out) shift equally, and
           variance is bias-invariant, so bias never reaches the device.)
  phase 2: recompute the matmul (x stays SBUF-resident; the only engines
           that can read PSUM are DVE and ACT, both already saturated, so
           a spill pass would cost more than PE recompute) and apply
           relu(psum * A + B) in one ScalarE pass, PSUM -> SBUF(fp16).

Scheduling (the critical path is DVE's ~22us of serialized bn_stats — every
downstream stage of the last chunk appends to it):
  - PE emits ALL phase-1 matmuls with priority; phase-2 recompute megas are
    interleaved only where they fill PE gaps. This keeps stats(c+1) from
    waiting on coefs(c) (the round-2 mistake, worth ~6us of DVE idle).
  - The last chunk's phase 2 is fine-grained: per-mega stores, relus split
    between ACT (PSUM relu) and DVE (tensor_scalar mult/add + max pair) so
    the post-stats tail is ~2 megas long instead of 4.
  - Input DMAs split across both HWDGE queues (Sync: wd + chunks 0-1,
    Scalar: gb + chunks 2-3) so descriptor generation parallelizes; output
    stores ride Sync behind the inputs.
  - Tiny per-chunk coefficient muls run on Pool; sqrt(c) is emitted right
    before the relus that consume it, after aggr(c) is long done.

Accuracy: ~5e-4 rel L2 vs the fp32 reference (fp16 rounding of x, W and y;
BN statistics run in fp32 from the f32 PSUM accumulators).
"""

import numpy as np

import concourse.bass as bass
import concourse.tile as tile
from concourse import mybir
from concourse.bass_utils import run_bass_kernel_spmd

F32 = mybir.dt.float32
F16 = mybir.dt.float16

NCORES = 8
BATCH = 4096
DIM = 4096
DCORE = DIM // NCORES          # 512 features per core
CHUNKS = DCORE // 128          # 4 partition chunks (2 groups each)
BTILE = 512                    # matmul moving tile (one PSUM bank, fp32)
BTILES = BATCH // BTILE        # 8
MEGA = 1024                    # PSUM mega-tile free dim (2 banks, 2 matmuls)
MEGAS = BATCH // MEGA          # 4 per chunk
EPS = 1e-3

_MAX_WAITS = 1


def _split_multi_waits(nc: bass.Bass, max_waits: int = _MAX_WAITS) -> None:
    # The walrus build in this container rejects instructions carrying more
    # than one sync-wait command (any engine, any opcode). Hoist extra waits
    # onto same-engine NOPs inserted immediately before the instruction —
    # identical semantics, since the engine blocks on each wait in order.
    # Snapshot every block BEFORE creating any nop: the engine builders append
    # new instructions to the current (last) block as a side effect, and the
    # final wholesale reassignment below discards those spurious appends.
    snapshots = [
        (bb, list(bb.instructions)) for f in nc.m.functions for bb in f.blocks
    ]
    rebuilt = []
    for bb, insts in snapshots:
        new = []
        for ins in insts:
            si = getattr(ins, "sync_info", None)
            waits = list(si.on_wait) if si is not None and si.on_wait else []
            if len(waits) > max_waits:
                head = waits[:-max_waits]
                for i in range(0, len(head), max_waits):
                    nop = nc.engines[ins.engine].nop().ins
                    nop.sync_info = mybir.SyncInfo(
                        on_wait=head[i : i + max_waits], on_update=[]
                    )
                    new.append(nop)
                ins.sync_info = mybir.SyncInfo(
                    on_wait=waits[-max_waits:],
                    on_update=list(si.on_update or []),
                )
            new.append(ins)
        rebuilt.append((bb, new))
    for bb, new in rebuilt:
        bb.instructions = new


def _build_nc() -> bass.Bass:
    nc = bass.Bass()
    xT = nc.dram_tensor("xT", [128, CHUNKS, BATCH], F16, kind="ExternalInput")
    wd = nc.dram_tensor("wd", [128, CHUNKS, 128], F16, kind="ExternalInput")
    gb = nc.dram_tensor("gb", [128, 2 * CHUNKS], F32, kind="ExternalInput")
    yT = nc.dram_tensor("yT", [128, CHUNKS, BATCH], F16, kind="ExternalOutput")

    with tile.TileContext(nc) as tc:
        with (
            tc.tile_pool(name="singles", bufs=1) as singles,
            tc.tile_pool(name="stats", bufs=1) as statp,
            tc.tile_pool(name="psum1", bufs=4, space="PSUM") as psum1,
            tc.tile_pool(name="psum2", bufs=2, space="PSUM") as psum2,
            tc.tile_pool(name="y", bufs=3) as ypool,
        ):
            # x streams on the Scalar HWDGE queue (first bytes ~0.7us after
            # main); wd/gb ride Sync in parallel, ahead of the stores.
            xsb = singles.tile([128, CHUNKS, BATCH], F16)
            wsb = singles.tile([128, CHUNKS, 128], F16)
            gbsb = singles.tile([128, 2 * CHUNKS], F32)
            for h in range(2):
                sl = bass.ds(h * (BATCH // 2), BATCH // 2)
                nc.scalar.dma_start(xsb[:, 0, sl], xT[:, 0, sl])
            for c in range(1, CHUNKS):
                nc.scalar.dma_start(xsb[:, c, :], xT[:, c, :])
            nc.sync.dma_start(wsb[:], wd[:, :, :])
            nc.sync.dma_start(gbsb[:], gb[:, :])
            epsb = singles.tile([128, 1], F32)
            nc.vector.memset(epsb[:], EPS)

            stats = statp.tile([128, CHUNKS, BTILES, 6], F32)
            mv = statp.tile([128, CHUNKS, 2], F32)
            coefA = statp.tile([128, CHUNKS], F32)
            coefB = statp.tile([128, CHUNKS], F32)
            tmp = statp.tile([128, CHUNKS], F32)
            tmp2 = statp.tile([128, CHUNKS], F32)

            def one_matmul(ps, os, c: int, t: int):
                nc.tensor.matmul(
                    ps[:, os],
                    lhsT=wsb[:, c, :],
                    rhs=xsb[:, c, bass.ds(t * BTILE, BTILE)],
                    start=True, stop=True,
                )

            def p1_tiles(c: int, lo: int, hi: int):
                for t in range(lo, hi):
                    ps = psum1.tile([128, BTILE], F32, tag="ps1")
                    one_matmul(ps, slice(None), c, t)
                    nc.vector.bn_stats(stats[:, c, t, :], ps[:, :])
                if hi == BTILES:
                    nc.vector.bn_aggr(mv[:, c, :], stats[:, c, :, :])

            def coefs(c: int):
                nc.scalar.activation(
                    tmp[:, c : c + 1], mv[:, c, 1:2],
                    mybir.ActivationFunctionType.Sqrt,
                    bias=epsb[:], scale=1.0,
                )
                nc.vector.reciprocal(tmp[:, c : c + 1], tmp[:, c : c + 1])
                nc.gpsimd.tensor_mul(
                    coefA[:, c : c + 1], tmp[:, c : c + 1], gbsb[:, c : c + 1]
                )
                nc.gpsimd.tensor_mul(
                    tmp2[:, c : c + 1], mv[:, c, 0:1], coefA[:, c : c + 1]
                )
                nc.gpsimd.tensor_sub(
                    coefB[:, c : c + 1],
                    gbsb[:, CHUNKS + c : CHUNKS + c + 1],
                    tmp2[:, c : c + 1],
                )

            yts: dict[int, object] = {}

            def p2_mega(c: int, m: int, dve: bool = False):
                # One phase-2 mega: 2 recompute matmuls + fused BN-affine
                # relu. Megas m0/m1 (m2/m3) share a [128, 2048] staging tile
                # so stores stay big. dve=True drains via a DVE
                # tensor_scalar pair instead of ACT (used to parallelize the
                # post-stats tail of the last chunk).
                if m % 2 == 0:
                    yts[c] = ypool.tile(
                        [128, 2 * MEGA], F16, tag="yt", name=f"yt_{c}_{m}"
                    )
                yt = yts[c]
                ys = yt[:, bass.ds((m % 2) * MEGA, MEGA)]
                ps = psum2.tile([128, MEGA], F32, tag="ps2")
                for q in range(MEGA // BTILE):
                    one_matmul(
                        ps, bass.ds(q * BTILE, BTILE), c,
                        m * (MEGA // BTILE) + q,
                    )
                if dve:
                    nc.vector.tensor_scalar(
                        ys, ps[:],
                        coefA[:, c : c + 1], coefB[:, c : c + 1],
                        mybir.AluOpType.mult, mybir.AluOpType.add,
                    )
                    nc.vector.tensor_scalar_max(ys, ys, 0.0)
                else:
                    nc.scalar.activation(
                        ys, ps[:],
                        mybir.ActivationFunctionType.Relu,
                        bias=coefB[:, c : c + 1], scale=coefA[:, c : c + 1],
                    )

            def store2(c: int, half: int):
                nc.sync.dma_start(
                    yT[:, c, bass.ds(half * 2 * MEGA, 2 * MEGA)], yts[c][:]
                )

            # Emission order = per-engine program order. The serialized DVE
            # stats chain (32 x ~0.7us) is the critical path: phase-1 tiles
            # flow to DVE without coef-latency in between (the round-2
            # mistake), while phase-2 megas are woven between phase-1
            # half-chunks purely as PE gap-filler (they never block DVE).
            # coefs(c) is emitted right before p2(c, 0) so ACT's in-order
            # queue sees sqrt(c) only after the last relu of chunk c-1.
            p1_tiles(0, 0, 8)
            p1_tiles(1, 0, 4)
            coefs(0)
            p2_mega(0, 0)
            p2_mega(0, 1)
            store2(0, 0)
            p1_tiles(1, 4, 8)
            p1_tiles(2, 0, 4)
            p2_mega(0, 2)
            p1_tiles(2, 4, 8)
            p2_mega(0, 3)
            store2(0, 1)
            coefs(1)
            p1_tiles(3, 0, 4)
            p2_mega(1, 0)
            p2_mega(1, 1)
            store2(1, 0)
            p1_tiles(3, 4, 8)
            p2_mega(1, 2)
            p2_mega(1, 3)
            store2(1, 1)
            coefs(2)
            p2_mega(2, 0)
            p2_mega(2, 1)
            store2(2, 0)
            p2_mega(2, 2)
            p2_mega(2, 3)
            store2(2, 1)
            coefs(3)
            # Last chunk: per-mega stores and ACT/DVE alternating drains so
            # the tail after the final bn_stats is two megas, not four.
            for m in range(MEGAS):
                p2_mega(3, m, dve=(m % 2 == 1))
                nc.sync.dma_start(
                    yT[:, 3, bass.ds(m * MEGA, MEGA)],
                    yts[3][:, bass.ds((m % 2) * MEGA, MEGA)],
                )
    _split_multi_waits(nc)
    return nc


_NC_CACHE: bass.Bass | None = None


def _get_nc() -> bass.Bass:
    global _NC_CACHE
    if _NC_CACHE is None:
        _NC_CACHE = _build_nc()
    return _NC_CACHE


def _make_in_maps(x, weight, gamma, beta):
    x16 = x.astype(np.float16)
    in_maps = []
    for c in range(NCORES):
        sl = slice(c * DCORE, (c + 1) * DCORE)
        # [p, ch, b] layout: partition p of chunk ch holds feature ch*128+p.
        xdev = np.ascontiguousarray(
            x16[:, sl].T.reshape(CHUNKS, 128, BATCH).transpose(1, 0, 2)
        )
        # Per 128-row chunk: [[w_{2g}, 0], [0, w_{2g+1}]] block-diagonal.
        wdc = np.zeros((DCORE, 128), np.float32)
        for g in range(DCORE // 64):
            r = slice(c * DCORE + g * 64, c * DCORE + (g + 1) * 64)
            col = (g % 2) * 64
            wdc[g * 64 : (g + 1) * 64, col : col + 64] = weight[r, r]
        wdev = np.ascontiguousarray(
            wdc.reshape(CHUNKS, 128, 128).transpose(1, 0, 2)
        ).astype(np.float16)
        gbdev = np.empty((128, 2 * CHUNKS), np.float32)
        gbdev[:, :CHUNKS] = gamma[sl].reshape(CHUNKS, 128).T
        gbdev[:, CHUNKS:] = beta[sl].reshape(CHUNKS, 128).T
        in_maps.append({"xT": xdev, "wd": wdev, "gb": gbdev})
    return in_maps


def kernel(x, weight, bias, gamma, beta, **_run_kwargs) -> np.ndarray:
    x = np.asarray(x, np.float32)
    weight = np.asarray(weight, np.float32)
    gamma = np.asarray(gamma, np.float32)
    beta = np.asarray(beta, np.float32)
    # bias is algebraically irrelevant: BN subtracts the batch mean, which
    # absorbs any constant per-feature shift, and variance is shift-invariant.

    nc = _get_nc()
    res = run_bass_kernel_spmd(
        nc, _make_in_maps(x, weight, gamma, beta),
        core_ids=list(range(NCORES)), **_run_kwargs,
    )
    out = np.empty((BATCH, DIM), np.float32)
    for c, r in enumerate(res.results):
        yc = r["yT"].transpose(1, 0, 2).reshape(DCORE, BATCH)
        out[:, c * DCORE : (c + 1) * DCORE] = yc.T.astype(np.float32)
    kernel.last_results = res
    return out


# revision 15
# speedup vs baseline: 1.1531x; 1.0086x over previous
"""Block-diagonal masked dense + BatchNorm(train) + ReLU on 8 TRN2 NeuronCores.

Math: out = x @ (W * blockdiag_mask) + bias; BN over batch; relu.
The mask keeps 64 diagonal blocks of shape [64 in, 64 out]. Group g only
couples x[:, 64g:64g+64] to out[:, 64g:64g+64].

Sharding: groups are split across cores (8 groups per core). Each core owns a
disjoint 512-column slice of both input and output features, so the matmul and
the per-feature batch statistics are fully core-local (no collectives).

Per-core device program (all shapes hardcoded, fp16 I/O — the 2e-2 harness
gate leaves ~40x margin over fp16's ~5e-4 rounding error, and halving the
bytes halves the HBM-bound runtime):
  inputs:  xT [128, 4, 4096] fp16 (x slice transposed + chunk-permuted on
           host so every DMA is 128 contiguous rows), wd [128, 4, 128] fp16
           (per 128-row chunk a 2x2 block-diagonal of two 64x64 group
           blocks, host-permuted), gb [128, 8] f32 (gamma | beta chunks)
  output:  yT [128, 4, 4096] fp16 (host transposes back and upcasts)
  phase 1: per chunk c and batch tile t: one K=128 fp16 matmul
           (block-diagonal zeros kill cross-group terms); DVE
           bn_stats/bn_aggr give mean/var per output feature.
  coefs:   A = gamma * rsqrt(var + eps); B = beta - mean * A.
           (bias cancels in BN: out and mean(out) shift equally, and
           variance is bias-invariant, so bias never reaches the device.)
  phase 2: recompute the matmul (x stays SBUF-resident; the only engines
           that can read PSUM are DVE and ACT, both already saturated, so
           a spill pass would cost more than PE recompute) and apply
           relu(psum * A + B) in one ScalarE pass, PSUM -> SBUF(fp16).

Scheduling (the critical path is DVE's ~22us of serialized bn_stats — every
downstream stage of the last chunk appends to it):
  - PE emits ALL phase-1 matmuls with priority; phase-2 recompute megas are
    interleaved only where they fill PE gaps. This keeps stats(c+1) from
    waiting on coefs(c) (the round-2 mistake, worth ~6us of DVE idle).
  - The last chunk's phase 2 is fine-grained: per-mega stores, relus split
    between ACT (PSUM relu) and DVE (tensor_scalar mult/add + max pair) so
    the post-stats tail is ~2 megas long instead of 4.
  - Input DMAs split across both HWDGE queues (Sync: wd + chunks 0-1,
    Scalar: gb + chunks 2-3) so descriptor generation parallelizes; output
    stores ride Sync behind the inputs.
  - Tiny per-chunk coefficient muls run on Pool; sqrt(c) is emitted right
    before the relus that consume it, after aggr(c) is long done.

Accuracy: ~5e-4 rel L2 vs the fp32 reference (fp16 rounding of x, W and y;
BN statistics run in fp32 from the f32 PSUM accumulators).
"""

import numpy as np

import concourse.bass as bass
import concourse.tile as tile
from concourse import mybir
from concourse.bass_utils import run_bass_kernel_spmd

F32 = mybir.dt.float32
F16 = mybir.dt.float16

NCORES = 8
BATCH = 4096
DIM = 4096
DCORE = DIM // NCORES          # 512 features per core
CHUNKS = DCORE // 128          # 4 partition chunks (2 groups each)
BTILE = 512                    # matmul moving tile (one PSUM bank, fp32)
BTILES = BATCH // BTILE        # 8
MEGA = 1024                    # PSUM mega-tile free dim (2 banks, 2 matmuls)
MEGAS = BATCH // MEGA          # 4 per chunk
EPS = 1e-3

_MAX_WAITS = 1


def _split_multi_waits(nc: bass.Bass, max_waits: int = _MAX_WAITS) -> None:
    # The walrus build in this container rejects instructions carrying more
    # than one sync-wait command (any engine, any opcode). Hoist extra waits
    # onto same-engine NOPs inserted immediately before the instruction —
    # identical semantics, since the engine blocks on each wait in order.
    # Snapshot every block BEFORE creating any nop: the engine builders append
    # new instructions to the current (last) block as a side effect, and the
    # final wholesale reassignment below discards those spurious appends.
    snapshots = [
        (bb, list(bb.instructions)) for f in nc.m.functions for bb in f.blocks
    ]
    rebuilt = []
    for bb, insts in snapshots:
        new = []
        for ins in insts:
            si = getattr(ins, "sync_info", None)
            waits = list(si.on_wait) if si is not None and si.on_wait else []
            if len(waits) > max_waits:
                head = waits[:-max_waits]
                for i in range(0, len(head), max_waits):
                    nop = nc.engines[ins.engine].nop().ins
                    nop.sync_info = mybir.SyncInfo(
                        on_wait=head[i : i + max_waits], on_update=[]
                    )
                    new.append(nop)
                ins.sync_info = mybir.SyncInfo(
                    on_wait=waits[-max_waits:],
                    on_update=list(si.on_update or []),
                )
            new.append(ins)
        rebuilt.append((bb, new))
    for bb, new in rebuilt:
        bb.instructions = new


def _build_nc() -> bass.Bass:
    nc = bass.Bass()
    xT = nc.dram_tensor("xT", [128, CHUNKS, BATCH], F16, kind="ExternalInput")
    wd = nc.dram_tensor("wd", [128, CHUNKS, 128], F16, kind="ExternalInput")
    gb = nc.dram_tensor("gb", [128, 2 * CHUNKS], F32, kind="ExternalInput")
    yT = nc.dram_tensor("yT", [128, CHUNKS, BATCH], F16, kind="ExternalOutput")

    with tile.TileContext(nc) as tc:
        with (
            tc.tile_pool(name="singles", bufs=1) as singles,
            tc.tile_pool(name="stats", bufs=1) as statp,
            tc.tile_pool(name="psum1", bufs=4, space="PSUM") as psum1,
            tc.tile_pool(name="psum2", bufs=2, space="PSUM") as psum2,
            tc.tile_pool(name="y", bufs=3) as ypool,
        ):
            # x streams on the Scalar HWDGE queue (first bytes ~0.7us after
            # main); wd/gb ride Sync in parallel, ahead of the stores.
            xsb = singles.tile([128, CHUNKS, BATCH], F16)
            wsb = singles.tile([128, CHUNKS, 128], F16)
            gbsb = singles.tile([128, 2 * CHUNKS], F32)
            for h in range(2):
                sl = bass.ds(h * (BATCH // 2), BATCH // 2)
                nc.scalar.dma_start(xsb[:, 0, sl], xT[:, 0, sl])
            for c in range(1, CHUNKS):
                nc.scalar.dma_start(xsb[:, c, :], xT[:, c, :])
            nc.sync.dma_start(wsb[:], wd[:, :, :])
            nc.sync.dma_start(gbsb[:], gb[:, :])
            epsb = singles.tile([128, 1], F32)
            nc.vector.memset(epsb[:], EPS)

            stats = statp.tile([128, CHUNKS, BTILES, 6], F32)
            mv = statp.tile([128, CHUNKS, 2], F32)
            coefA = statp.tile([128, CHUNKS], F32)
            coefB = statp.tile([128, CHUNKS], F32)
            tmp = statp.tile([128, CHUNKS], F32)
            tmp2 = statp.tile([128, CHUNKS], F32)

            def one_matmul(ps, os, c: int, t: int):
                nc.tensor.matmul(
                    ps[:, os],
                    lhsT=wsb[:, c, :],
                    rhs=xsb[:, c, bass.ds(t * BTILE, BTILE)],
                    start=True, stop=True,
                )

            def p1_tiles(c: int, lo: int, hi: int):
                for t in range(lo, hi):
                    ps = psum1.tile([128, BTILE], F32, tag="ps1")
                    one_matmul(ps, slice(None), c, t)
                    nc.vector.bn_stats(stats[:, c, t, :], ps[:, :])
                if hi == BTILES:
                    nc.vector.bn_aggr(mv[:, c, :], stats[:, c, :, :])

            def coefs(c: int):
                nc.scalar.activation(
                    tmp[:, c : c + 1], mv[:, c, 1:2],
                    mybir.ActivationFunctionType.Sqrt,
                    bias=epsb[:], scale=1.0,
                )
                nc.vector.reciprocal(tmp[:, c : c + 1], tmp[:, c : c + 1])
                nc.gpsimd.tensor_mul(
                    coefA[:, c : c + 1], tmp[:, c : c + 1], gbsb[:, c : c + 1]
                )
                nc.gpsimd.tensor_mul(
                    tmp2[:, c : c + 1], mv[:, c, 0:1], coefA[:, c : c + 1]
                )
                nc.gpsimd.tensor_sub(
                    coefB[:, c : c + 1],
                    gbsb[:, CHUNKS + c : CHUNKS + c + 1],
                    tmp2[:, c : c + 1],
                )

            yts: dict[int, object] = {}

            def p2_mega(c: int, m: int, dve: bool = False):
                # One phase-2 mega: 2 recompute matmuls + fused BN-affine
                # relu. Megas m0/m1 (m2/m3) share a [128, 2048] staging tile
                # so stores stay big. dve=True drains via a DVE
                # tensor_scalar pair instead of ACT (used to parallelize the
                # post-stats tail of the last chunk).
                if m % 2 == 0 or c not in yts:
                    yts[c] = ypool.tile(
                        [128, 2 * MEGA], F16, tag="yt", name=f"yt_{c}_{m}"
                    )
                yt = yts[c]
                ys = yt[:, bass.ds((m % 2) * MEGA, MEGA)]
                ps = psum2.tile([128, MEGA], F32, tag="ps2")
                for q in range(MEGA // BTILE):
                    one_matmul(
                        ps, bass.ds(q * BTILE, BTILE), c,
                        m * (MEGA // BTILE) + q,
                    )
                if dve:
                    nc.vector.tensor_scalar(
                        ys, ps[:],
                        coefA[:, c : c + 1], coefB[:, c : c + 1],
                        mybir.AluOpType.mult, mybir.AluOpType.add,
                    )
                    nc.vector.tensor_scalar_max(ys, ys, 0.0)
                else:
                    nc.scalar.activation(
                        ys, ps[:],
                        mybir.ActivationFunctionType.Relu,
                        bias=coefB[:, c : c + 1], scale=coefA[:, c : c + 1],
                    )

            def store2(c: int, half: int):
                nc.sync.dma_start(
                    yT[:, c, bass.ds(half * 2 * MEGA, 2 * MEGA)], yts[c][:]
                )

            # Emission order = per-engine program order. The serialized DVE
            # stats chain (32 x ~0.7us) is the critical path: phase-1 tiles
            # flow to DVE without coef-latency in between (the round-2
            # mistake), while phase-2 megas are woven between phase-1
            # half-chunks purely as PE gap-filler (they never block DVE).
            # coefs(c) is emitted right before p2(c, 0) so ACT's in-order
            # queue sees sqrt(c) only after the last relu of chunk c-1.
            p1_tiles(0, 0, 8)
            p1_tiles(1, 0, 4)
            coefs(0)
            p2_mega(0, 0)
            p2_mega(0, 1)
            store2(0, 0)
            p1_tiles(1, 4, 8)
            p1_tiles(2, 0, 4)
            p2_mega(0, 2)
            p1_tiles(2, 4, 8)
            p2_mega(0, 3)
            store2(0, 1)
            coefs(1)
            p1_tiles(3, 0, 4)
            p2_mega(1, 0)
            p2_mega(1, 1)
            store2(1, 0)
            p1_tiles(3, 4, 8)
            p2_mega(1, 2)
            p2_mega(1, 3)
            store2(1, 1)
            coefs(2)
            p2_mega(2, 0)
            p2_mega(2, 1)
            store2(2, 0)
            p2_mega(2, 2)
            p2_mega(2, 3)
            store2(2, 1)
            coefs(3)
            # Last chunk: per-mega stores and ACT/DVE alternating drains so
            # the tail after the final bn_stats is two megas, not four.
            for m in range(MEGAS):
                p2_mega(3, m, dve=(m % 2 == 1))
                nc.sync.dma_start(
                    yT[:, 3, bass.ds(m * MEGA, MEGA)],
                    yts[3][:, bass.ds((m % 2) * MEGA, MEGA)],
                )
    _split_multi_waits(nc)
    return nc


_NC_CACHE: bass.Bass | None = None


def _get_nc() -> bass.Bass:
    global _NC_CACHE
    if _NC_CACHE is None:
        _NC_CACHE = _build_nc()
    return _NC_CACHE


def _make_in_maps(x, weight, gamma, beta):
    x16 = x.astype(np.float16)
    in_maps = []
    for c in range(NCORES):
        sl = slice(c * DCORE, (c + 1) * DCORE)
        # [p, ch, b] layout: partition p of chunk ch holds feature ch*128+p.
        xdev = np.ascontiguousarray(
            x16[:, sl].T.reshape(CHUNKS, 128, BATCH).transpose(1, 0, 2)
        )
        # Per 128-row chunk: [[w_{2g}, 0], [0, w_{2g+1}]] block-diagonal.
        wdc = np.zeros((DCORE, 128), np.float32)
        for g in range(DCORE // 64):
            r = slice(c * DCORE + g * 64, c * DCORE + (g + 1) * 64)
            col = (g % 2) * 64
            wdc[g * 64 : (g + 1) * 64, col : col + 64] = weight[r, r]
        wdev = np.ascontiguousarray(
            wdc.reshape(CHUNKS, 128, 128).transpose(1, 0, 2)
        ).astype(np.float16)
        gbdev = np.empty((128, 2 * CHUNKS), np.float32)
        gbdev[:, :CHUNKS] = gamma[sl].reshape(CHUNKS, 128).T
        gbdev[:, CHUNKS:] = beta[sl].reshape(CHUNKS, 128).T
        in_maps.append({"xT": xdev, "wd": wdev, "gb": gbdev})
    return in_maps


def kernel(x, weight, bias, gamma, beta, **_run_kwargs) -> np.ndarray:
    x = np.asarray(x, np.float32)
    weight = np.asarray(weight, np.float32)
    gamma = np.asarray(gamma, np.float32)
    beta = np.asarray(beta, np.float32)
    # bias is algebraically irrelevant: BN subtracts the batch mean, which
    # absorbs any constant per-feature shift, and variance is shift-invariant.

    nc = _get_nc()
    res = run_bass_kernel_spmd(
        nc, _make_in_maps(x, weight, gamma, beta),
        core_ids=list(range(NCORES)), **_run_kwargs,
    )
    out = np.empty((BATCH, DIM), np.float32)
    for c, r in enumerate(res.results):
        yc = r["yT"].transpose(1, 0, 2).reshape(DCORE, BATCH)
        out[:, c * DCORE : (c + 1) * DCORE] = yc.T.astype(np.float32)
    kernel.last_results = res
    return out
